# revision 7
# baseline (speedup 1.0000x reference)
"""Trainium2 Bass kernel for out = x * w (column-wise scale).

x: [16384, 4096] f32, w: [4096] f32 -> out[i, j] = x[i, j] * w[j].

Data-parallel across 8 NeuronCores: each core handles a [2048, 4096] row
shard of x; w is replicated. Per core the shard streams through SBUF as
16 tiles of [128, 4096] (2 MiB DMAs), multiplied in place on the vector
engine, and stored back.

Hardware model (measured on this part):
- Each NeuronCore has 16 SDMA engines at ~26.5 GB/s each (16 KiB packet
  in ~610 ns); they round-robin across whichever DMA queues have work.
  Per-NC DMA bandwidth is therefore ~425 GB/s no matter how many queues
  are used, and loads/stores share it. 67.1 MB of traffic -> ~158 us of
  pure transfer time; the framework preamble adds ~8 us. The kernel's
  job is to keep at least one queue non-empty at every instant.
- Cast-during-DMA (bf16) does NOT reduce engine time (charged on the
  f32 side), so everything stays f32 and exact.

Design notes (raw Bass, no Tile framework):
- Three DMA rings (gpsimd SWDGE + sync/SP and scalar/ACT HWDGE) carry
  contiguous tile ranges: q0 tiles 0-5, q1 tiles 6-10, q2 tiles 11-15,
  each with a PRIVATE 3-slot SBUF ring. A queue's ring is L L L S0 L S1
  L S2 ...: loads run eagerly, stores chase the DVE multiplies, and by
  the time the rings drain every queue holds a backlog of ready stores
  (their multiplies long done), so the tail has no serial
  load->multiply->store chain.
- Slot reuse needs no store semaphores: load k overwrites slot k%3 only
  after store k-3 was issued on the SAME queue (ring program order), and
  per-engine FIFO descriptor order within a queue makes the store's
  SBUF read complete before the load's SBUF write. The store's
  wait_ge(dve_sem) also precedes the load's issue, covering the DVE
  read. Stores still carry a then_inc on a dead semaphore because
  walrus rejects DGE ops without sync info; nothing waits on it.
- All tile loads gate on the 16 KiB w row (first on the sync ring), so
  w lands in ~2 us at full rate and the PE broadcast (rank-1 matmul
  ones[128,1] @ w[1,4096] into PSUM) finishes by ~13 us; a dummy matmul
  absorbs PE cold-start. The ~2 us of idle this costs up front buys
  stores that start flowing at ~16 us instead of ~40 us.
- The DVE multiplies tiles in ARRIVAL order (round-robin across the
  three queues: 0,6,11,1,7,12,...), in place, at half-tile grain so the
  first multiply only needs the first 4 PSUM banks.
"""

import sys

for _p in ("/opt/trn_rl_repo",):
    if _p not in sys.path:
        sys.path.insert(0, _p)

from contextlib import ExitStack

import numpy as np

import concourse.bass as bass
import concourse.mybir as mybir
from concourse.bass_utils import run_bass_kernel_spmd

ROWS = 16384
SIZE = 4096
N_CORES = 8
ROWS_PER_CORE = ROWS // N_CORES  # 2048
P = 128                          # SBUF partitions
N_TILES = ROWS_PER_CORE // P     # 16 tiles of [128, 4096]
QSLOTS = 3                       # private SBUF ring depth per queue
NQ = 3                           # DMA queues (gpsimd, sync, scalar)

# Contiguous tile ranges per queue; q0 slightly heavier.
QTILES = [list(range(0, 6)), list(range(6, 11)), list(range(11, 16))]
# DVE processes tiles in arrival order: local index k across queues.
DVE_ORDER = [
    t
    for k in range(max(len(q) for q in QTILES))
    for q in QTILES
    if k < len(q)
    for t in [q[k]]
]
DVE_POS = {t: i for i, t in enumerate(DVE_ORDER)}

_nc_cache = None


def _build() -> bass.Bass:
    f32 = mybir.dt.float32
    nc = bass.Bass()
    x = nc.declare_dram_parameter("x", [ROWS_PER_CORE, SIZE], f32, isOutput=False)
    w = nc.declare_dram_parameter("w", [SIZE], f32, isOutput=False)
    y = nc.declare_dram_parameter("y", [ROWS_PER_CORE, SIZE], f32, isOutput=True)

    with ExitStack() as ctx:
        w_row = ctx.enter_context(nc.sbuf_tensor([1, SIZE], f32))
        ones_t = ctx.enter_context(nc.sbuf_tensor([1, P], f32))
        psum_w = ctx.enter_context(nc.psum_tensor([P, SIZE], f32))
        tbuf = ctx.enter_context(nc.sbuf_tensor([P, NQ * QSLOTS * SIZE], f32))
        w_sem = ctx.enter_context(nc.semaphore("w_sem"))
        ones_sem = ctx.enter_context(nc.semaphore("ones_sem"))
        pe_sem = ctx.enter_context(nc.semaphore("pe_sem"))
        dve_sem = ctx.enter_context(nc.semaphore("dve_sem"))
        st_sem = ctx.enter_context(nc.semaphore("st_sem"))
        in_sems = [
            [
                ctx.enter_context(nc.semaphore(f"in_sem{q}_{s}"))
                for s in range(QSLOTS)
            ]
            for q in range(NQ)
        ]
        block = ctx.enter_context(nc.Block())

        HALF = SIZE // 2

        def slot(qid, s):
            a = qid * QSLOTS + s
            return tbuf[:, a * SIZE : (a + 1) * SIZE]

        def emit_queue(q: bass.BassEngine, qid: int):
            if qid == 1:
                # 16 KiB w row first on the sync HWDGE ring; everything
                # else gates on it so it lands at full rate.
                q.dma_start(out=w_row[:], in_=w[None, :]).then_inc(w_sem, 16)
            q.wait_ge(w_sem, 16)
            tiles = QTILES[qid]
            li = si = 0
            while li < len(tiles) or si < len(tiles):
                while li < len(tiles) and (si >= len(tiles) or li < si + QSLOTS):
                    j = tiles[li]
                    s = li % QSLOTS
                    q.dma_start(
                        out=slot(qid, s), in_=x[j * P : (j + 1) * P, :]
                    ).then_inc(in_sems[qid][s], 16)
                    li += 1
                if si < len(tiles):
                    i = tiles[si]
                    q.wait_ge(dve_sem, 2 * DVE_POS[i] + 2)
                    q.dma_start(
                        out=y[i * P : (i + 1) * P, :], in_=slot(qid, si % QSLOTS)
                    ).then_inc(st_sem, 16)  # walrus requires sync info; no waiter
                    si += 1

        @block.gpsimd
        def _(g: bass.BassEngine):
            emit_queue(g, 0)

        @block.sync
        def _(s: bass.BassEngine):
            emit_queue(s, 1)

        @block.scalar
        def _(s: bass.BassEngine):
            emit_queue(s, 2)

        MM_N = 512  # one PSUM bank of f32 per matmul

        @block.tensor
        def _(t: bass.BassEngine):
            t.wait_ge(ones_sem, 1)
            # dummy matmul absorbs PE cold-start before w arrives
            t.matmul(
                psum_w[:, 0:P], ones_t[:], ones_t[:],
                start=True, stop=True,
            )
            t.wait_ge(w_sem, 16)
            for b in range(SIZE // MM_N):
                # psum_w[p, n] = ones[0, p] * w_row[0, n] — partition bcast
                t.matmul(
                    psum_w[:, b * MM_N : (b + 1) * MM_N],
                    ones_t[:],
                    w_row[:, b * MM_N : (b + 1) * MM_N],
                    start=True,
                    stop=True,
                ).then_inc(pe_sem, 1)

        HALF_BANKS = HALF // MM_N  # matmuls needed per half of psum_w

        @block.vector
        def _(v: bass.BassEngine):
            v.memset(ones_t[:], 1.0).then_inc(ones_sem, 1)
            uses = [[0] * QSLOTS for _ in range(NQ)]
            first = True
            for t in DVE_ORDER:
                qid = next(q for q in range(NQ) if t in QTILES[q])
                k = QTILES[qid].index(t)
                s = k % QSLOTS
                uses[qid][s] += 1
                v.wait_ge(in_sems[qid][s], 16 * uses[qid][s])
                for h in range(2):
                    if first:
                        v.wait_ge(pe_sem, HALF_BANKS * (h + 1))
                    c0, c1 = h * HALF, (h + 1) * HALF
                    sl = slot(qid, s)
                    v.tensor_mul(
                        sl[:, c0:c1], sl[:, c0:c1], psum_w[:, c0:c1]
                    ).then_inc(dve_sem, 1)
                first = False

    return nc


def _run(x: np.ndarray, w: np.ndarray, **spmd_kwargs):
    global _nc_cache
    if _nc_cache is None:
        _nc_cache = _build()
    x = np.ascontiguousarray(x, dtype=np.float32)
    w = np.ascontiguousarray(w, dtype=np.float32)
    in_maps = [
        {"x": x[i * ROWS_PER_CORE : (i + 1) * ROWS_PER_CORE], "w": w}
        for i in range(N_CORES)
    ]
    return run_bass_kernel_spmd(_nc_cache, in_maps, list(range(N_CORES)), **spmd_kwargs)


def kernel(x: np.ndarray, w: np.ndarray) -> np.ndarray:
    res = _run(x, w)
    return np.concatenate([res.results[i]["y"] for i in range(N_CORES)], axis=0)


# revision 8
# speedup vs baseline: 1.1833x; 1.1833x over previous
"""Trainium2 Bass kernel for out = x * w (column-wise scale).

x: [16384, 4096] f32, w: [4096] f32 -> out[i, j] = x[i, j] * w[j].

Data-parallel across 8 NeuronCores: each core handles a [2048, 4096] row
shard of x; w is replicated. Per core the shard streams through SBUF as
8 blocks of [256, 4096] (4 MiB DMAs, 32 KiB contiguous per partition on
both the DRAM and SBUF side), multiplied in place on the vector engine,
and stored back.

Hardware model (measured on this part):
- Per NeuronCore, 16 SDMA engines (~27 GB/s each; 16 KiB packet in
  ~610 ns) serve every DMA queue round-robin, so per-NC DMA bandwidth is
  ~425-470 GB/s total no matter how many queues are used; loads and
  stores share it. 67.1 MB of mandatory traffic -> ~145-160 us floor
  plus the ~8 us fixed framework preamble. More queues do NOT add
  bandwidth (a 3-queue variant measured slower); two queues with big
  descriptors and minimal semaphore traffic get closest to the floor.
- Cast-during-DMA (bf16 SBUF side) does not reduce SDMA engine time
  (charged on the f32 side), so everything stays f32 and bit-exact.

Design notes (raw Bass, no Tile framework):
- Two DMA rings: gpsimd (SWDGE) carries even blocks, sync (HWDGE) odd
  blocks, each block's load AND store on the same ring. 17 DMAs total
  (8 loads + 8 stores + w) — half the descriptor/semaphore overhead of
  a per-tile schedule, with 32 KiB contiguous runs per partition.
- Slot reuse needs no store-completion semaphores: block j reuses slot
  j%6 only after the store of block j-6 was issued on the SAME ring
  (6%2==0), and per-engine FIFO descriptor order makes the store's SBUF
  read complete before the load's SBUF write; the store's
  wait_ge(dve_sem) also precedes the load's issue, covering the DVE
  read. Stores carry a then_inc on a dead semaphore only because walrus
  rejects DGE ops without sync info.
- w is fetched once as a 16 KiB row and broadcast across partitions
  on-chip with a rank-1 PE matmul into PSUM (ones[128,1] @ w[1,4096]);
  multiplies read w straight from PSUM, keeping the w image out of
  SBUF. A dummy matmul absorbs PE cold-start. w_row and the ones vector
  live inside slot 5's partition-0 tail; the load of block 5 — the
  first writer of that slot — gates on pe_sem>=8, by which point the PE
  has consumed both (this frees the SBUF needed for six 32 KiB slots).
- Block 0 multiplies its first chunk at half grain so the first multiply
  only needs the first 4 PSUM banks while the PE fills the rest.
"""

import sys

for _p in ("/opt/trn_rl_repo",):
    if _p not in sys.path:
        sys.path.insert(0, _p)

from contextlib import ExitStack

import numpy as np

import concourse.bass as bass
import concourse.mybir as mybir
from concourse.bass_utils import run_bass_kernel_spmd

ROWS = 16384
SIZE = 4096
N_CORES = 8
ROWS_PER_CORE = ROWS // N_CORES   # 2048
P = 128
BLK = 2
BSIZE = BLK * SIZE                # 8192 per partition per block
N_BLOCKS = ROWS_PER_CORE // (P * BLK)  # 8
SLOTS = 6                         # 6 % 2 == 0 -> reuse stays on-queue
NQ = 2

_nc_cache = None


def _build() -> bass.Bass:
    f32 = mybir.dt.float32
    nc = bass.Bass()
    x = nc.declare_dram_parameter("x", [ROWS_PER_CORE, SIZE], f32, isOutput=False)
    w = nc.declare_dram_parameter("w", [SIZE], f32, isOutput=False)
    y = nc.declare_dram_parameter("y", [ROWS_PER_CORE, SIZE], f32, isOutput=True)

    with ExitStack() as ctx:
        tbuf = ctx.enter_context(nc.sbuf_tensor([P, SLOTS * BSIZE], f32))
        psum_w = ctx.enter_context(nc.psum_tensor([P, SIZE], f32))
        w_sem = ctx.enter_context(nc.semaphore("w_sem"))
        ones_sem = ctx.enter_context(nc.semaphore("ones_sem"))
        pe_sem = ctx.enter_context(nc.semaphore("pe_sem"))
        dve_sem = ctx.enter_context(nc.semaphore("dve_sem"))
        st_sem = ctx.enter_context(nc.semaphore("st_sem"))
        in_sems = [
            ctx.enter_context(nc.semaphore(f"in_sem{a}")) for a in range(SLOTS)
        ]
        block = ctx.enter_context(nc.Block())

        w_row = tbuf[0:1, 5 * BSIZE + BSIZE - SIZE : 6 * BSIZE]
        ones_t = tbuf[0:1, 5 * BSIZE + BSIZE - SIZE - P : 5 * BSIZE + BSIZE - SIZE]

        def slot(a):
            return tbuf[:, a * BSIZE : (a + 1) * BSIZE]

        RPB = P * BLK  # 256 rows per block

        def emit_queue(q: bass.BassEngine, qid: int):
            if qid == 1:
                # 16 KiB w row first on the sync HWDGE ring
                q.dma_start(out=w_row, in_=w[None, :]).then_inc(w_sem, 16)
            blocks = list(range(qid, N_BLOCKS, NQ))
            li = si = 0
            while li < len(blocks) or si < len(blocks):
                while li < len(blocks) and (
                    si >= len(blocks) or blocks[li] < blocks[si] + SLOTS
                ):
                    j = blocks[li]
                    if j == 5:
                        # slot 5 holds w_row/ones until the PE consumed them
                        q.wait_ge(pe_sem, 8)
                    q.dma_start(
                        out=slot(j % SLOTS), in_=x[j * RPB : (j + 1) * RPB, :]
                    ).then_inc(in_sems[j % SLOTS], 16)
                    li += 1
                if si < len(blocks):
                    i = blocks[si]
                    q.wait_ge(dve_sem, 2 * i + 2)
                    q.dma_start(
                        out=y[i * RPB : (i + 1) * RPB, :], in_=slot(i % SLOTS)
                    ).then_inc(st_sem, 16)  # walrus requires sync info; no waiter
                    si += 1

        @block.gpsimd
        def _(g: bass.BassEngine):
            emit_queue(g, 0)

        @block.sync
        def _(s: bass.BassEngine):
            emit_queue(s, 1)

        MM_N = 512  # one PSUM bank of f32 per matmul

        @block.tensor
        def _(t: bass.BassEngine):
            t.wait_ge(ones_sem, 1)
            # dummy matmul absorbs PE cold-start before w arrives
            t.matmul(psum_w[:, 0:P], ones_t, ones_t, start=True, stop=True)
            t.wait_ge(w_sem, 16)
            for b in range(SIZE // MM_N):
                # psum_w[p, n] = ones[0, p] * w_row[0, n] — partition bcast
                t.matmul(
                    psum_w[:, b * MM_N : (b + 1) * MM_N],
                    ones_t,
                    w_row[:, b * MM_N : (b + 1) * MM_N],
                    start=True,
                    stop=True,
                ).then_inc(pe_sem, 1)

        HALF = SIZE // 2
        HALF_BANKS = HALF // MM_N

        @block.vector
        def _(v: bass.BassEngine):
            v.memset(ones_t, 1.0).then_inc(ones_sem, 1)
            for i in range(N_BLOCKS):
                a = i % SLOTS
                v.wait_ge(in_sems[a], 16 * (i // SLOTS + 1))
                sl = slot(a)
                if i == 0:
                    # chunk 0 at half grain (h0 needs only 4 PSUM banks);
                    # inc on h1 + chunk 1 so dve_sem hits 2 exactly when
                    # the block is fully multiplied
                    for h in range(2):
                        v.wait_ge(pe_sem, HALF_BANKS * (h + 1))
                        c0, c1 = h * HALF, (h + 1) * HALF
                        mm = v.tensor_mul(
                            sl[:, c0:c1], sl[:, c0:c1], psum_w[:, c0:c1]
                        )
                        if h == 1:
                            mm.then_inc(dve_sem, 1)
                    v.tensor_mul(
                        sl[:, SIZE : 2 * SIZE], sl[:, SIZE : 2 * SIZE],
                        psum_w[:, :],
                    ).then_inc(dve_sem, 1)
                else:
                    for c in range(BLK):
                        c0, c1 = c * SIZE, (c + 1) * SIZE
                        v.tensor_mul(
                            sl[:, c0:c1], sl[:, c0:c1], psum_w[:, :]
                        ).then_inc(dve_sem, 1)

    return nc


def _run(x: np.ndarray, w: np.ndarray, **spmd_kwargs):
    global _nc_cache
    if _nc_cache is None:
        _nc_cache = _build()
    x = np.ascontiguousarray(x, dtype=np.float32)
    w = np.ascontiguousarray(w, dtype=np.float32)
    in_maps = [
        {"x": x[i * ROWS_PER_CORE : (i + 1) * ROWS_PER_CORE], "w": w}
        for i in range(N_CORES)
    ]
    return run_bass_kernel_spmd(_nc_cache, in_maps, list(range(N_CORES)), **spmd_kwargs)


def kernel(x: np.ndarray, w: np.ndarray) -> np.ndarray:
    res = _run(x, w)
    return np.concatenate([res.results[i]["y"] for i in range(N_CORES)], axis=0)
